# revision 36
# baseline (speedup 1.0000x reference)
"""BERT-base encoder layer on 8 Trainium2 NeuronCores (Bass/Tile).

Sharding: data-parallel over batch. Full inputs [32, 512, 768] split into 8
shards of 4 batches (2048 tokens); every core runs the same NEFF on its shard
(SPMD, no collectives); host concatenates the outputs.

All GEMMs run on the PE in bf16 with fp32 PSUM accumulation; softmax and
layernorm statistics run in fp32. 1/sqrt(dk) is folded into Wq on the host.

Attention is computed in TRANSPOSED score layout: scoresT[kv, q] = K^T Q with
kv on partitions. This makes the additive attention mask a per-partition bias
applied for free inside the exp activation, and it removes the PE transpose
of the probabilities entirely (exp(scoresT) feeds P@V directly as the moving
operand). Softmax denominators come free out of the P@V matmul by augmenting
V with 64 columns of ones ([V|ones] / [ones|V] per head pair): the same
matmul that produces the 64 attention rows produces 64 broadcast rows of the
denominator in the other half of the PSUM bank. The reciprocal runs on the
Scalar engine as exp(-ln(s)) (the DVE reciprocal is a slow multi-pass op),
a stride-0-free-dim DMA moves each head's 1/s row into the partition range
of its attention rows, and a DVE multiply applies the normalization.

Phase B (O-proj, LN1, FFN, LN2) is software-pipelined across batches so the
PE always has independent matmul work during GELU/layernorm tails, and all
GEMM PSUM banks are released after a single copy/STT pass.
"""

import os
import numpy as np
import ml_dtypes

B, S, E, H, DK, FF = 32, 512, 768, 12, 64, 3072
NCORES = 8
BL = B // NCORES          # batches per core = 4
T = BL * S                # tokens per core = 2048
EPS = 1e-12
MASK_NEG = -87.0          # stays inside exp-table range; exp() == 0 in fp32

_CACHE = {}


def _bf(a):
    return np.ascontiguousarray(np.asarray(a, np.float32).astype(ml_dtypes.bfloat16))


def _build(flags):
    import concourse.bass as bass
    import concourse.bacc as bacc
    import concourse.mybir as mybir
    import concourse.tile as tile
    from contextlib import ExitStack

    (use_bq, use_bk, use_bv, use_bo, use_bso, use_bi, use_bout,
     use_g1, use_b1, use_g2, use_b2) = flags

    AF = mybir.ActivationFunctionType
    OP = mybir.AluOpType
    AX = mybir.AxisListType
    BF16 = mybir.dt.bfloat16
    F32 = mybir.dt.float32

    nc = bacc.Bacc("TRN2", target_bir_lowering=False)

    # Both Exp and Ln are used in the attention inner loop. The default
    # act-table chooser assigns them different table sets, forcing a ~1.3us
    # ACT_TABLE_LOAD on every switch. Steer the chooser to the one set that
    # holds both (natural_log_exp_and_others) by hiding Exp/Ln in the other
    # sets this instance sees; set ids keep their act_info.json positions.
    import types as _types
    from concourse.hw_specs import get_activation_tables as _gat

    def _patched_act_loads(self):
        import bass_rust as _br
        has_act = any(isinstance(i, mybir.InstActivation)
                      for blk in self.main_func.blocks
                      for i in blk.instructions)
        if not has_act:
            return
        AFT = mybir.ActivationFunctionType
        items = []
        for name, funcs in _gat(self.m.arch).items():
            if name != "natural_log_exp_and_others":
                funcs = {f for f in funcs if f not in (AFT.Exp, AFT.Ln)}
            items.append((name, funcs))
        _br.insert_act_table_loads(self, items)

    nc.insert_act_table_loads = _types.MethodType(_patched_act_loads, nc)

    d_x = nc.dram_tensor("x", (E, T), BF16, kind="ExternalInput")
    d_wq = nc.dram_tensor("wq", (E, E), BF16, kind="ExternalInput")
    d_wk = nc.dram_tensor("wk", (E, E), BF16, kind="ExternalInput")
    d_wv = nc.dram_tensor("wv", (E, E), BF16, kind="ExternalInput")
    d_wo = nc.dram_tensor("wo", (E, E), BF16, kind="ExternalInput")
    d_wso = nc.dram_tensor("wso", (E, E), BF16, kind="ExternalInput")
    d_wi = nc.dram_tensor("wi", (E, FF), BF16, kind="ExternalInput")
    d_wout = nc.dram_tensor("wout", (FF, E), BF16, kind="ExternalInput")
    # per-(batch, kv-tile) additive mask bias column: [128, BL*NT_B] fp32
    d_mcol = nc.dram_tensor("mcol", (128, BL * (S // 128)), mybir.dt.float32,
                            kind="ExternalInput")
    d_id = nc.dram_tensor("ident", (128, 128), BF16, kind="ExternalInput")
    d_ones = nc.dram_tensor("onesrow", (1, 512), BF16, kind="ExternalInput")
    # bias rows: 0=bq/8, 1=bk, 2=bv, 3=bo, 4=bso, 5=bout, 6=bi (full FF width)
    d_brow = nc.dram_tensor("brow", (7, FF), BF16, kind="ExternalInput")
    d_bic = nc.dram_tensor("bicol", (128, FF // 128), F32, kind="ExternalInput")
    # gamma1 | beta1 | gamma2 | beta2, each [128, 768] partition-broadcast
    d_gb = nc.dram_tensor("gb", (128, 4 * E), F32, kind="ExternalInput")
    d_out = nc.dram_tensor("out", (T, E), F32, kind="ExternalOutput")

    KT_E = E // 128    # 6
    NT_B = S // 128    # 4
    FT = FF // 128     # 24
    HP = H // 2        # 6

    need_gb = use_g1 or use_b1 or use_g2 or use_b2
    need_brow = use_bq or use_bk or use_bv or use_bo or use_bso or use_bout

    with ExitStack() as ctx:
        tc = ctx.enter_context(tile.TileContext(nc))

        p_mm = ctx.enter_context(tc.tile_pool(name="p_mm", bufs=2, space="PSUM"))

        c_pool = ctx.enter_context(tc.tile_pool(name="consts", bufs=1))
        xt_pool = ctx.enter_context(tc.tile_pool(name="xt", bufs=BL * KT_E))
        att_pool = ctx.enter_context(tc.tile_pool(name="attp", bufs=BL * KT_E + 4))

        wo_pool = ctx.enter_context(tc.tile_pool(name="wo", bufs=KT_E))
        WO = []
        ident = c_pool.tile_from(d_id[:, :], name="ident")
        ones = c_pool.tile_from(d_ones[:, :], name="ones")
        brow = c_pool.tile_from(d_brow[:, :], name="brow") if need_brow else None
        gb = c_pool.tile_from(d_gb[:, :], name="gb") if need_gb else None

        XT = {}    # (b, kt) -> [128, S] bf16, feature-major x
        ATT = {}   # (b, kt) -> [128, S] bf16, feature-major normalized attention

        # ================= superphase A: x^T, QKV, attention =================
        with ExitStack() as sa:
            a_pool = sa.enter_context(tc.tile_pool(name="a_consts", bufs=1))
            wq_pool = sa.enter_context(tc.tile_pool(name="wq", bufs=KT_E))
            wk_pool = sa.enter_context(tc.tile_pool(name="wk", bufs=KT_E))
            wv_pool = sa.enter_context(tc.tile_pool(name="wv", bufs=KT_E))
            qt_pool = sa.enter_context(tc.tile_pool(name="qt", bufs=KT_E + 3))
            kt_pool = sa.enter_context(tc.tile_pool(name="kt", bufs=KT_E + 3))
            va_pool = sa.enter_context(tc.tile_pool(name="va", bufs=NT_B + 1))
            # scoresT psum: [128, 1024] fp32 = 2 banks, both hh halves
            p_sc = sa.enter_context(tc.tile_pool(name="p_sc", bufs=2,
                                                 space="PSUM"))
            p_av = sa.enter_context(tc.tile_pool(name="p_av", bufs=2,
                                                 space="PSUM"))
            ex_pool = sa.enter_context(tc.tile_pool(name="ex", bufs=7))
            r_pool = sa.enter_context(tc.tile_pool(name="rp", bufs=3))

            mcol = a_pool.tile_from(d_mcol[:, :], name="mcol",
                                    forced_dma_engine=mybir.EngineType.SP)
            WQ = [wq_pool.tile_from(d_wq[k * 128:(k + 1) * 128, :], name="wqt")
                  for k in range(KT_E)]
            WK = [wk_pool.tile_from(d_wk[k * 128:(k + 1) * 128, :], name="wkt",
                                    forced_dma_engine=mybir.EngineType.SP)
                  for k in range(KT_E)]
            WV = []

            QTb, KTb, VAb, EXH = {}, {}, {}, {}

            def emit_xt(b):
                t0 = b * S
                for kt in range(KT_E):
                    XT[(b, kt)] = xt_pool.tile([128, S], BF16, name="xtt",
                                               tag="xt")
                    nc.gpsimd.dma_start(
                        XT[(b, kt)][:, :],
                        d_x[kt * 128:(kt + 1) * 128, t0:t0 + S])

            def emit_qk_group(b, idx):
                # idx 0..5: Q projection tile et=idx; 6..11: K tile et=idx-6
                if b not in QTb:
                    QTb[b], KTb[b] = [None] * KT_E, [None] * KT_E
                if idx < KT_E:
                    Wt, dstl, pool, ub, brx, tg = WQ, QTb[b], qt_pool, use_bq, 0, "qt"
                    et = idx
                else:
                    Wt, dstl, pool, ub, brx, tg = WK, KTb[b], kt_pool, use_bk, 1, "kt"
                    et = idx - KT_E
                ps = p_mm.tile([128, S], F32, name="qkps", tag="mm")
                for k in range(KT_E):
                    nc.tensor.matmul(
                        ps[:, :], Wt[k][:, et * 128:(et + 1) * 128],
                        XT[(b, k)][:, :],
                        start=(k == 0), stop=(k == KT_E - 1 and not ub))
                if ub:
                    nc.tensor.matmul(
                        ps[:, :], brow[brx:brx + 1, et * 128:(et + 1) * 128],
                        ones[0:1, 0:S], start=False, stop=True)
                dstl[et] = pool.tile([128, S], BF16, name="qkt", tag=tg)
                nc.vector.tensor_copy(dstl[et][:, :], ps[:, :])

            def emit_vaug(b):
                if b == 0:
                    WV.extend(wv_pool.tile_from(
                        d_wv[k * 128:(k + 1) * 128, :], name="wvt")
                        for k in range(KT_E))
                    WO.extend(wo_pool.tile_from(
                        d_wo[k * 128:(k + 1) * 128, :], name="wot")
                        for k in range(KT_E))
                # per 256-col head-pair block: [V_e(64) | ones(128) | V_o(64)]
                VA = VAb[b] = [None] * NT_B
                for tt in range(NT_B):
                    VA[tt] = va_pool.tile([128, HP * 256], BF16, name="vat",
                                          tag="va")
                    va3 = VA[tt].rearrange("p (blk c) -> p blk c", c=256)
                    nc.vector.memset(va3[:, :, 64:192], 1.0)
                    for ec, n in ((0, 512), (512, 256)):
                        ps = (p_mm.tile([128, 512], F32, name="vps", tag="mm")
                              if n == 512 else
                              p_mm.tile([128, 256], F32, name="vps2", tag="mm"))
                        for k in range(KT_E):
                            nc.tensor.matmul(
                                ps[:, :n], XT[(b, k)][:, tt * 128:(tt + 1) * 128],
                                WV[k][:, ec:ec + n],
                                start=(k == 0), stop=(k == KT_E - 1 and not use_bv))
                        if use_bv:
                            nc.tensor.matmul(
                                ps[:, :n], ones[0:1, 0:128],
                                brow[2:3, ec:ec + n], start=False, stop=True)
                        ps3 = ps.rearrange("p (h c) -> p h c", c=128)
                        nh_pairs = n // 128
                        blk0 = ec // 128
                        nc.vector.tensor_copy(
                            va3[:, blk0:blk0 + nh_pairs, 0:64],
                            ps3[:, 0:nh_pairs, 0:64])
                        nc.vector.tensor_copy(
                            va3[:, blk0:blk0 + nh_pairs, 192:256],
                            ps3[:, 0:nh_pairs, 64:128])

            def emit_scores(b, hp):
                QT, KTt = QTb[b], KTb[b]
                EXs = [None] * NT_B
                for jt in range(NT_B):
                    psc = p_sc.tile([128, 1024], F32, name="scps", tag="sc")
                    for hh in range(2):
                        o = hh * 64
                        nc.tensor.matmul(
                            psc[:, hh * 512:(hh + 1) * 512],
                            KTt[hp][o:o + 64, jt * 128:(jt + 1) * 128],
                            QT[hp][o:o + 64, :], start=True, stop=True)
                    ex = ex_pool.tile([128, 1024], BF16, name="ext", tag="ex")
                    mc = b * NT_B + jt
                    nc.scalar.activation(ex[:, :], psc[:, :], AF.Exp,
                                         bias=mcol[:, mc:mc + 1])
                    EXs[jt] = ex
                EXH[(b, hp)] = EXs

            def emit_av(b, hp):
                # P@V: [V|ones] -> att rows 0:64 + denom rows 64:128 (even)
                #      [ones|V] -> denom rows 0:64 + att rows 64:128 (odd)
                VA = VAb[b]
                EXs = EXH.pop((b, hp))
                psA = p_av.tile([128, 512], F32, name="avA", tag="av")
                psB = p_av.tile([128, 512], F32, name="avB", tag="av")
                for jt in range(NT_B):
                    nc.tensor.matmul(
                        psA[:, :], VA[jt][:, hp * 256:hp * 256 + 128],
                        EXs[jt][:, 0:512],
                        start=(jt == 0), stop=(jt == NT_B - 1))
                    nc.tensor.matmul(
                        psB[:, :], VA[jt][:, hp * 256 + 128:hp * 256 + 256],
                        EXs[jt][:, 512:1024],
                        start=(jt == 0), stop=(jt == NT_B - 1))
                # Release the PSUM banks fast: copy both banks to SBUF bf16
                # immediately; the normalize chain then runs from SBUF.
                uA = r_pool.tile([128, 512], BF16, name="uA", tag="uA")
                uB = r_pool.tile([128, 512], BF16, name="uB", tag="uB")
                nc.vector.tensor_copy(uA[:, :], psA[:, :])
                nc.vector.tensor_copy(uB[:, :], psB[:, :])
                # 1/s via ACT: ln(s) then exp(-x), BOTH before the broadcast
                # DMA so the Scalar queue never sits on a DMA wait.
                ra = r_pool.tile([128, 512], F32, name="ra", tag="ra")
                nc.scalar.activation(ra[64:128, :], uA[64:128, :], AF.Ln)
                nc.scalar.activation(ra[0:64, :], uB[0:64, :], AF.Ln)
                rp = r_pool.tile([128, 512], BF16, name="rp", tag="rp")
                nc.scalar.activation(rp[64:128, :], ra[64:128, :], AF.Exp,
                                     scale=-1.0)
                nc.scalar.activation(rp[0:64, :], ra[0:64, :], AF.Exp,
                                     scale=-1.0)
                rc = r_pool.tile([128, 512], BF16, name="rc", tag="rc")
                # DMA broadcast: one row -> 64 partitions (free-dim 0-stride
                # src; partition-stride-0 APs are rejected by lowering and
                # the gpsimd partition_broadcast ucode is broken on HW).
                nc.sync.dma_start(
                    bass.AP(rc.tensor, rc.offset,
                            [[512, 64], [1, 1], [1, 512]]),
                    bass.AP(rp.tensor, rp.offset + 64 * 512,
                            [[512, 1], [0, 64], [1, 512]]))
                nc.sync.dma_start(
                    bass.AP(rc.tensor, rc.offset + 64 * 512,
                            [[512, 64], [1, 1], [1, 512]]),
                    bass.AP(rp.tensor, rp.offset,
                            [[512, 1], [0, 64], [1, 512]]))
                att = att_pool.tile([128, S], BF16, name="attt", tag="attT")
                nc.vector.scalar_tensor_tensor(
                    att[0:64, :], uA[0:64, :], 1.0, rc[0:64, :],
                    op0=OP.mult, op1=OP.mult)
                nc.vector.scalar_tensor_tensor(
                    att[64:128, :], uB[64:128, :], 1.0, rc[64:128, :],
                    op0=OP.mult, op1=OP.mult)
                ATT[(b, hp)] = att

            # batch b's attention cycles carry batch b+1's Q/K projection
            # groups (emitted BEFORE each av so the PE FIFO has independent
            # work while the exp chain drains), and batch b+1's V_aug after
            # the last av (its V tiles only free then).
            emit_xt(0)
            for i in range(2 * KT_E):
                emit_qk_group(0, i)
            emit_vaug(0)
            for b in range(BL):
                if b + 1 < BL:
                    emit_xt(b + 1)
                emit_scores(b, 0)
                for hp in range(HP):
                    if hp + 1 < HP:
                        emit_scores(b, hp + 1)
                    if b + 1 < BL:
                        emit_qk_group(b + 1, 2 * hp)
                        emit_qk_group(b + 1, 2 * hp + 1)
                    emit_av(b, hp)
                if b + 1 < BL:
                    emit_vaug(b + 1)

        # ============ superphase B: O-proj, SelfOutput LN, FFN, LN ===========
        with ExitStack() as sb:
            b_pool = sb.enter_context(tc.tile_pool(name="b_consts", bufs=1))
            wso_pool = sb.enter_context(tc.tile_pool(name="wso", bufs=KT_E))
            wi_pool = sb.enter_context(tc.tile_pool(name="wi", bufs=KT_E))
            wout_pool = sb.enter_context(tc.tile_pool(name="wout", bufs=FT))
            h_pool = sb.enter_context(tc.tile_pool(name="h", bufs=2 * NT_B))
            ht_pool = sb.enter_context(tc.tile_pool(name="ht", bufs=2))
            fft_pool = sb.enter_context(tc.tile_pool(name="fft", bufs=FT))
            sq_pool = sb.enter_context(tc.tile_pool(name="sq", bufs=1))
            rs_pool = sb.enter_context(tc.tile_pool(name="rsd", bufs=3))
            out_pool = sb.enter_context(tc.tile_pool(name="outp", bufs=2, space="SBUF"))
            t_pool = sb.enter_context(tc.tile_pool(name="sb_s", bufs=4))
            p_mmb = sb.enter_context(tc.tile_pool(name="p_mmb", bufs=6,
                                                  space="PSUM"))

            bic = b_pool.tile_from(d_bic[:, :], name="bic") if use_bi else None
            WSO = [wso_pool.tile_from(d_wso[k * 128:(k + 1) * 128, :],
                                      name="wsot") for k in range(KT_E)]
            WI = [wi_pool.tile_from(d_wi[k * 128:(k + 1) * 128, :], name="wit")
                  for k in range(KT_E)]
            WOUT = [wout_pool.tile_from(d_wout[f * 128:(f + 1) * 128, :],
                                        name="woutt") for f in range(FT)]

            def layernorm(chunks, h_dst, gcol, use_g, use_bb, resid=None):
                """chunks: [(psum_ap, col0, n)]; h_dst: [128, E] fp32 out.
                resid: parallel list of sbuf fp32 APs added to psum first."""
                rtile = rs_pool.tile([128, E], BF16, name="rt", tag="rsd")
                if resid is not None:
                    for (ps, c0, n), rext in zip(chunks, resid):
                        nc.vector.scalar_tensor_tensor(
                            rtile[:, c0:c0 + n], ps, 1.0, rext,
                            op0=OP.mult, op1=OP.add)
                else:
                    # copy PSUM chunks out immediately so the GEMM psum bank
                    # is released after one DVE pass instead of being held
                    # through the whole layernorm chain
                    for (ps, c0, n) in chunks:
                        nc.vector.tensor_copy(rtile[:, c0:c0 + n], ps)
                srcs = [(rtile[:, c0:c0 + n], c0, n) for (_, c0, n) in chunks]
                s1 = t_pool.tile([128, 1], F32, name="s1", tag="s1")
                s1b = t_pool.tile([128, 1], F32, name="s1b", tag="s1b")
                nc.vector.reduce_sum(s1[:, :], srcs[0][0], axis=AX.X)
                nc.vector.reduce_sum(s1b[:, :], srcs[1][0], axis=AX.X)
                mu_n = t_pool.tile([128, 1], F32, name="mun", tag="mun")
                # mu_n = -(s1 + s1b)/E
                tmp = t_pool.tile([128, 1], F32, name="tmps", tag="tmps")
                nc.vector.scalar_tensor_tensor(
                    tmp[:, :], s1[:, :], 1.0, s1b[:, :], op0=OP.mult, op1=OP.add)
                nc.vector.tensor_scalar_mul(mu_n[:, :], tmp[:, :], -1.0 / E)
                ss = t_pool.tile([128, 1], F32, name="ssa", tag="ssa", bufs=16)
                ssb = t_pool.tile([128, 1], F32, name="ssb", tag="ssb", bufs=16)
                for (src, c0, n), acc in zip(srcs, (ss, ssb)):
                    sq = sq_pool.tile([128, 512], BF16, name="sqt", tag="sq")
                    nc.scalar.activation(sq[:, :n], src, AF.Square,
                                         accum_out=acc[:, :])
                # var = (ss+ssb)/E - mu^2 ; rstd = 1/sqrt(var + eps)
                musq = t_pool.tile([128, 1], F32, name="musq", tag="musq")
                nc.vector.scalar_tensor_tensor(
                    musq[:, :], mu_n[:, :], 1.0, mu_n[:, :],
                    op0=OP.mult, op1=OP.mult)
                veps = t_pool.tile([128, 1], F32, name="veps", tag="veps")
                nc.vector.scalar_tensor_tensor(
                    veps[:, :], ss[:, :], 1.0, ssb[:, :],
                    op0=OP.mult, op1=OP.add)
                veps2 = t_pool.tile([128, 1], F32, name="veps2", tag="veps2")
                nc.vector.tensor_scalar(
                    veps2[:, :], veps[:, :], 1.0 / E, EPS,
                    op0=OP.mult, op1=OP.add)
                veps3 = t_pool.tile([128, 1], F32, name="veps3", tag="veps3")
                nc.vector.scalar_tensor_tensor(
                    veps3[:, :], musq[:, :], -1.0, veps2[:, :],
                    op0=OP.mult, op1=OP.add)
                sd = t_pool.tile([128, 1], F32, name="sd", tag="sd")
                nc.scalar.sqrt(sd[:, :], veps3[:, :])
                rstd = t_pool.tile([128, 1], F32, name="rstd", tag="rstd")
                nc.vector.reciprocal(rstd[:, :], sd[:, :])
                for (src, c0, n) in srcs:
                    nc.vector.tensor_scalar(
                        h_dst[:, c0:c0 + n], src, mu_n[:, :], rstd[:, :],
                        op0=OP.add, op1=OP.mult)
                if use_g:
                    nc.vector.scalar_tensor_tensor(
                        h_dst[:, :], h_dst[:, :], 1.0,
                        gb[:, gcol * E:(gcol + 1) * E], op0=OP.mult, op1=OP.mult)
                if use_bb:
                    nc.vector.scalar_tensor_tensor(
                        h_dst[:, :], h_dst[:, :], 1.0,
                        gb[:, (gcol + 2) * E:(gcol + 3) * E],
                        op0=OP.mult, op1=OP.add)

            HHT, HTT, FFT, HTR = {}, {}, {}, {}

            def emit_oso(b):
                # ---- O-projection + residual -> xa (feature-major bf16) ----
                xa = [None] * KT_E
                for et in range(KT_E):
                    ps = p_mmb.tile([128, S], F32, name="ops", tag="mm")
                    for k in range(KT_E):
                        nc.tensor.matmul(
                            ps[:, :], WO[k][:, et * 128:(et + 1) * 128],
                            ATT[(b, k)][:, :],
                            start=(k == 0), stop=(k == KT_E - 1 and not use_bo))
                    if use_bo:
                        nc.tensor.matmul(
                            ps[:, :], brow[3:4, et * 128:(et + 1) * 128],
                            ones[0:1, 0:S], start=False, stop=True)
                    xa[et] = att_pool.tile([128, S], BF16, name="xat", tag="attT")
                    nc.vector.scalar_tensor_tensor(
                        xa[et][:, :], ps[:, :], 1.0, XT[(b, et)][:, :],
                        op0=OP.mult, op1=OP.add)

                # ---- SelfOutput GEMM + LN1 -> h (token-major fp32), hT ----
                hh_t = [None] * NT_B
                hT = ht_pool.tile([128, KT_E * S], BF16, name="htt", tag="ht")

                def emit_htrans(tt):
                    tps = [p_mm.tile([128, 512], BF16, name="htp", tag="mm")
                           for _ in range(2)]
                    for et in range(KT_E):
                        sl = tps[et // 4][:, (et % 4) * 128:(et % 4 + 1) * 128]
                        nc.tensor.transpose(
                            sl, hh_t[tt][:, et * 128:(et + 1) * 128],
                            ident[:, :])
                    for et in range(KT_E):
                        sl = tps[et // 4][:, (et % 4) * 128:(et % 4 + 1) * 128]
                        nc.vector.tensor_copy(
                            hT[:, et * S + tt * 128:et * S + (tt + 1) * 128], sl)

                # skewed: h-transposes of tile tt are emitted after the
                # SO GEMM of tile tt+1, so the PE never waits on LN1
                for tt in range(NT_B):
                    ch = []
                    for ec, n in ((0, 512), (512, 256)):
                        ps = (p_mm.tile([128, 512], F32, name="sops", tag="mm")
                              if n == 512 else
                              p_mm.tile([128, 256], F32, name="sops2", tag="mm"))
                        for k in range(KT_E):
                            nc.tensor.matmul(
                                ps[:, :n], xa[k][:, tt * 128:(tt + 1) * 128],
                                WSO[k][:, ec:ec + n],
                                start=(k == 0),
                                stop=(k == KT_E - 1 and not use_bso))
                        if use_bso:
                            nc.tensor.matmul(
                                ps[:, :n], ones[0:1, 0:128],
                                brow[4:5, ec:ec + n], start=False, stop=True)
                        ch.append((ps[:, :n], ec, n))
                    hh_t[tt] = h_pool.tile([128, E], BF16, name="hht", tag="h")
                    layernorm(ch, hh_t[tt], 0, use_g1, use_b1)
                HHT[b], HTT[b] = hh_t, hT
                HTR[b] = emit_htrans

            def emit_wi(b):
                # ---- FFN Wi + GELU (full 512-token chunk: N=512 GEMMs) ----
                hT = HTT.pop(b)
                ffT = [None] * FT
                for ft in range(FT):
                    ps = p_mmb.tile([128, 512], F32, name="fips", tag="mm")
                    for k in range(KT_E):
                        nc.tensor.matmul(
                            ps[:, :], WI[k][:, ft * 128:(ft + 1) * 128],
                            hT[:, k * S:k * S + 512],
                            start=(k == 0), stop=(k == KT_E - 1))
                    ffT[ft] = fft_pool.tile([128, 512], BF16, name="fftt",
                                            tag="fft")
                    if use_bi:
                        nc.scalar.activation(ffT[ft][:, :], ps[:, :],
                                             AF.Gelu,
                                             bias=bic[:, ft:ft + 1])
                    else:
                        nc.scalar.activation(ffT[ft][:, :], ps[:, :],
                                             AF.Gelu)
                FFT[b] = ffT

            def emit_wout(b, inter=None):
                # ---- Wout + residual + LN2 -> out ----
                t0 = b * S
                ffT = FFT.pop(b)
                hh_t = HHT.pop(b)
                for tt in range(NT_B):
                    if inter is not None and tt >= 1:
                        inter(tt - 1)
                    ch = []
                    for ec, n in ((0, 512), (512, 256)):
                        ps = (p_mmb.tile([128, 512], F32, name="wops",
                                         tag="mm") if n == 512 else
                              p_mmb.tile([128, 256], F32, name="wops2",
                                         tag="mm"))
                        for f in range(FT):
                            nc.tensor.matmul(
                                ps[:, :n],
                                ffT[f][:, tt * 128:(tt + 1) * 128],
                                WOUT[f][:, ec:ec + n],
                                start=(f == 0),
                                stop=(f == FT - 1 and not use_bout))
                        if use_bout:
                            nc.tensor.matmul(
                                ps[:, :n], ones[0:1, 0:128],
                                brow[5:6, ec:ec + n], start=False,
                                stop=True)
                        ch.append((ps[:, :n], ec, n))
                    otile = out_pool.tile([128, E], F32, name="ot",
                                          tag="outp")
                    resid = [hh_t[tt][:, ec:ec + n] for (_, ec, n) in ch]
                    layernorm(ch, otile, 1, use_g2, use_b2, resid=resid)
                    nc.gpsimd.dma_start(
                        d_out[t0 + tt * 128:t0 + (tt + 1) * 128, :],
                        otile[:, :])

            # software-pipelined: O/SO/LN1 of batch b+1 are emitted between
            # Wi(b) and Wout(b); the h-transposes of b+1 are interleaved
            # into Wout(b)'s token-tile loop so they land after their LN1
            # chains have completed, and the independent Wout work is never
            # queued behind an LN-stalled transpose.
            emit_oso(0)
            for tt in range(NT_B):
                HTR[0](tt)
            for b in range(BL):
                emit_wi(b)
                if b + 1 < BL:
                    emit_oso(b + 1)
                emit_wout(b)
                if b + 1 < BL:
                    for tt in range(NT_B):
                        HTR[b + 1](tt)
                    del HTR[b + 1]
    nc.compile()
    return nc


def _get_program(flags):
    key = ("prog", flags)
    if key not in _CACHE:
        _CACHE[key] = _build(flags)
    return _CACHE[key]


def kernel(x, mask, Wq, bq, Wk, bk, Wv, bv, Wo, bo,
           Wso, bso, gso, beso, Wi, bi, Wout, bout, gout, beout):
    from concourse.bass_utils import run_bass_kernel_spmd

    x = np.asarray(x, np.float32)
    mask = np.asarray(mask)
    sc = 1.0 / float(np.sqrt(np.float32(DK)))

    z = lambda a: not np.any(np.asarray(a))
    one = lambda a: bool(np.all(np.asarray(a) == 1.0))
    flags = (not z(bq), not z(bk), not z(bv), not z(bo), not z(bso),
             not z(bi), not z(bout),
             not one(gso), not z(beso), not one(gout), not z(beout))
    nc = _get_program(flags)

    wq_b = _bf(np.asarray(Wq, np.float32) * sc)
    wk_b, wv_b, wo_b, wso_b = _bf(Wk), _bf(Wv), _bf(Wo), _bf(Wso)
    wi_b, wout_b = _bf(Wi), _bf(Wout)
    identb = _bf(np.eye(128))
    onesr = _bf(np.ones((1, 512)))

    brow = np.zeros((7, FF), np.float32)
    brow[0, :E] = np.asarray(bq, np.float32) * sc
    for i, v in enumerate((bk, bv, bo, bso, bout)):
        brow[i + 1, :E] = v
    brow[6, :] = bi
    brow = _bf(brow)
    bicol = np.asarray(bi, np.float32).reshape(FF // 128, 128).T.copy()
    gbt = np.zeros((128, 4 * E), np.float32)
    for i, g in enumerate((gso, gout, beso, beout)):   # gamma1|gamma2|beta1|beta2
        gbt[:, i * E:(i + 1) * E] = np.broadcast_to(
            np.asarray(g, np.float32).reshape(1, E), (128, E))

    in_maps = []
    for c in range(NCORES):
        xs = np.ascontiguousarray(x[c * BL:(c + 1) * BL].reshape(T, E).T)
        ms = np.asarray(mask[c * BL:(c + 1) * BL]).reshape(BL, S)
        mb = np.where(ms == 0, np.float32(MASK_NEG), np.float32(0.0))
        # mcol[p, b*4+jt] = mask bias of kv position jt*128+p in batch b
        mcolv = np.ascontiguousarray(
            mb.reshape(BL, S // 128, 128).transpose(2, 0, 1).reshape(
                128, BL * (S // 128)).astype(np.float32))
        in_maps.append({
            "x": _bf(xs), "wq": wq_b, "wk": wk_b, "wv": wv_b, "wo": wo_b,
            "wso": wso_b, "wi": wi_b, "wout": wout_b, "mcol": mcolv,
            "ident": identb, "onesrow": onesr,
            "brow": brow, "bicol": bicol, "gb": gbt,
        })

    trace = os.environ.get("KERNEL_TRACE", "0") == "1"
    res = run_bass_kernel_spmd(nc, in_maps, core_ids=list(range(NCORES)),
                               trace=trace)
    if trace and res.exec_time_ns is not None:
        print(f"HW exec time: {res.exec_time_ns} ns")
        if res.instructions_and_trace is not None:
            print(f"trace: {res.instructions_and_trace[1]}")
    out = np.concatenate([r["out"].reshape(BL, S, E) for r in res.results],
                         axis=0)
    return np.ascontiguousarray(out.astype(np.float32))


# revision 37
# speedup vs baseline: 1.1951x; 1.1951x over previous
"""BERT-base encoder layer on 8 Trainium2 NeuronCores (Bass/Tile).

Sharding: data-parallel over batch. Full inputs [32, 512, 768] split into 8
shards of 4 batches (2048 tokens); every core runs the same NEFF on its shard
(SPMD, no collectives); host concatenates the outputs.

All GEMMs run on the PE in bf16 with fp32 PSUM accumulation; softmax and
layernorm statistics run in fp32. 1/sqrt(dk) is folded into Wq on the host.

Attention is computed in TRANSPOSED score layout: scoresT[kv, q] = K^T Q with
kv on partitions. This makes the additive attention mask a per-partition bias
applied for free inside the exp activation, and it removes the PE transpose
of the probabilities entirely (exp(scoresT) feeds P@V directly as the moving
operand). Softmax denominators come free out of the P@V matmul by augmenting
V with 64 columns of ones ([V|ones] / [ones|V] per head pair): the same
matmul that produces the 64 attention rows produces 64 broadcast rows of the
denominator in the other half of the PSUM bank. The reciprocal runs on the
Scalar engine as exp(-ln(s)) (the DVE reciprocal is a slow multi-pass op),
a stride-0-free-dim DMA moves each head's 1/s row into the partition range
of its attention rows, and a DVE multiply applies the normalization.

Phase B (O-proj, LN1, FFN, LN2) is software-pipelined across batches so the
PE always has independent matmul work during GELU/layernorm tails, and all
GEMM PSUM banks are released after a single copy/STT pass.
"""

import os
import numpy as np
import ml_dtypes

B, S, E, H, DK, FF = 32, 512, 768, 12, 64, 3072
NCORES = 8
BL = B // NCORES          # batches per core = 4
T = BL * S                # tokens per core = 2048
EPS = 1e-12
MASK_NEG = -87.0          # stays inside exp-table range; exp() == 0 in fp32

_CACHE = {}


def _bf(a):
    return np.ascontiguousarray(np.asarray(a, np.float32).astype(ml_dtypes.bfloat16))


def _build(flags):
    import concourse.bass as bass
    import concourse.bacc as bacc
    import concourse.mybir as mybir
    import concourse.tile as tile
    from contextlib import ExitStack

    (use_bq, use_bk, use_bv, use_bo, use_bso, use_bi, use_bout,
     use_g1, use_b1, use_g2, use_b2) = flags

    AF = mybir.ActivationFunctionType
    OP = mybir.AluOpType
    AX = mybir.AxisListType
    BF16 = mybir.dt.bfloat16
    F32 = mybir.dt.float32

    nc = bacc.Bacc("TRN2", target_bir_lowering=False)

    # Both Exp and Ln are used in the attention inner loop. The default
    # act-table chooser assigns them different table sets, forcing a ~1.3us
    # ACT_TABLE_LOAD on every switch. Steer the chooser to the one set that
    # holds both (natural_log_exp_and_others) by hiding Exp/Ln in the other
    # sets this instance sees; set ids keep their act_info.json positions.
    import types as _types
    from concourse.hw_specs import get_activation_tables as _gat

    def _patched_act_loads(self):
        import bass_rust as _br
        has_act = any(isinstance(i, mybir.InstActivation)
                      for blk in self.main_func.blocks
                      for i in blk.instructions)
        if not has_act:
            return
        AFT = mybir.ActivationFunctionType
        items = []
        for name, funcs in _gat(self.m.arch).items():
            if name != "natural_log_exp_and_others":
                funcs = {f for f in funcs if f not in (AFT.Exp, AFT.Ln)}
            items.append((name, funcs))
        _br.insert_act_table_loads(self, items)

    nc.insert_act_table_loads = _types.MethodType(_patched_act_loads, nc)

    d_x = nc.dram_tensor("x", (E, T), BF16, kind="ExternalInput")
    d_wq = nc.dram_tensor("wq", (E, E), BF16, kind="ExternalInput")
    d_wk = nc.dram_tensor("wk", (E, E), BF16, kind="ExternalInput")
    d_wv = nc.dram_tensor("wv", (E, E), BF16, kind="ExternalInput")
    d_wo = nc.dram_tensor("wo", (E, E), BF16, kind="ExternalInput")
    d_wso = nc.dram_tensor("wso", (E, E), BF16, kind="ExternalInput")
    d_wi = nc.dram_tensor("wi", (E, FF), BF16, kind="ExternalInput")
    d_wout = nc.dram_tensor("wout", (FF, E), BF16, kind="ExternalInput")
    # per-(batch, kv-tile) additive mask bias column: [128, BL*NT_B] fp32
    d_mcol = nc.dram_tensor("mcol", (128, BL * (S // 128)), mybir.dt.float32,
                            kind="ExternalInput")
    d_id = nc.dram_tensor("ident", (128, 128), BF16, kind="ExternalInput")
    d_ones = nc.dram_tensor("onesrow", (1, 512), BF16, kind="ExternalInput")
    # bias rows: 0=bq/8, 1=bk, 2=bv, 3=bo, 4=bso, 5=bout, 6=bi (full FF width)
    d_brow = nc.dram_tensor("brow", (7, FF), BF16, kind="ExternalInput")
    d_bic = nc.dram_tensor("bicol", (128, FF // 128), F32, kind="ExternalInput")
    # gamma1 | beta1 | gamma2 | beta2, each [128, 768] partition-broadcast
    d_gb = nc.dram_tensor("gb", (128, 4 * E), F32, kind="ExternalInput")
    d_out = nc.dram_tensor("out", (T, E), F32, kind="ExternalOutput")

    KT_E = E // 128    # 6
    NT_B = S // 128    # 4
    FT = FF // 128     # 24
    HP = H // 2        # 6

    need_gb = use_g1 or use_b1 or use_g2 or use_b2
    need_brow = use_bq or use_bk or use_bv or use_bo or use_bso or use_bout

    with ExitStack() as ctx:
        tc = ctx.enter_context(tile.TileContext(nc))

        p_mm = ctx.enter_context(tc.tile_pool(name="p_mm", bufs=2, space="PSUM"))

        c_pool = ctx.enter_context(tc.tile_pool(name="consts", bufs=1))
        xt_pool = ctx.enter_context(tc.tile_pool(name="xt", bufs=BL * KT_E))
        att_pool = ctx.enter_context(tc.tile_pool(name="attp", bufs=BL * KT_E + 4))

        wo_pool = ctx.enter_context(tc.tile_pool(name="wo", bufs=KT_E))
        WO = []
        ident = c_pool.tile_from(d_id[:, :], name="ident")
        ones = c_pool.tile_from(d_ones[:, :], name="ones")
        brow = c_pool.tile_from(d_brow[:, :], name="brow") if need_brow else None
        gb = c_pool.tile_from(d_gb[:, :], name="gb") if need_gb else None

        XT = {}    # (b, kt) -> [128, S] bf16, feature-major x
        ATT = {}   # (b, kt) -> [128, S] bf16, feature-major normalized attention

        # ================= superphase A: x^T, QKV, attention =================
        with ExitStack() as sa:
            a_pool = sa.enter_context(tc.tile_pool(name="a_consts", bufs=1))
            wq_pool = sa.enter_context(tc.tile_pool(name="wq", bufs=KT_E))
            wk_pool = sa.enter_context(tc.tile_pool(name="wk", bufs=KT_E))
            wv_pool = sa.enter_context(tc.tile_pool(name="wv", bufs=KT_E))
            qt_pool = sa.enter_context(tc.tile_pool(name="qt", bufs=KT_E + 3))
            kt_pool = sa.enter_context(tc.tile_pool(name="kt", bufs=KT_E + 3))
            va_pool = sa.enter_context(tc.tile_pool(name="va", bufs=NT_B + 1))
            # scoresT psum: [128, 1024] fp32 = 2 banks, both hh halves
            p_sc = sa.enter_context(tc.tile_pool(name="p_sc", bufs=2,
                                                 space="PSUM"))
            p_av = sa.enter_context(tc.tile_pool(name="p_av", bufs=2,
                                                 space="PSUM"))
            ex_pool = sa.enter_context(tc.tile_pool(name="ex", bufs=7))
            r_pool = sa.enter_context(tc.tile_pool(name="rp", bufs=3))

            mcol = a_pool.tile_from(d_mcol[:, :], name="mcol")
            WQ = [wq_pool.tile_from(d_wq[k * 128:(k + 1) * 128, :], name="wqt")
                  for k in range(KT_E)]
            WK = [wk_pool.tile_from(d_wk[k * 128:(k + 1) * 128, :], name="wkt")
                  for k in range(KT_E)]
            WV = []

            QTb, KTb, VAb, EXH = {}, {}, {}, {}

            def emit_xt(b):
                t0 = b * S
                for kt in range(KT_E):
                    XT[(b, kt)] = xt_pool.tile([128, S], BF16, name="xtt",
                                               tag="xt")
                    nc.gpsimd.dma_start(
                        XT[(b, kt)][:, :],
                        d_x[kt * 128:(kt + 1) * 128, t0:t0 + S])

            def emit_qk_group(b, idx):
                # idx 0..5: Q projection tile et=idx; 6..11: K tile et=idx-6
                if b not in QTb:
                    QTb[b], KTb[b] = [None] * KT_E, [None] * KT_E
                if idx < KT_E:
                    Wt, dstl, pool, ub, brx, tg = WQ, QTb[b], qt_pool, use_bq, 0, "qt"
                    et = idx
                else:
                    Wt, dstl, pool, ub, brx, tg = WK, KTb[b], kt_pool, use_bk, 1, "kt"
                    et = idx - KT_E
                ps = p_mm.tile([128, S], F32, name="qkps", tag="mm")
                for k in range(KT_E):
                    nc.tensor.matmul(
                        ps[:, :], Wt[k][:, et * 128:(et + 1) * 128],
                        XT[(b, k)][:, :],
                        start=(k == 0), stop=(k == KT_E - 1 and not ub))
                if ub:
                    nc.tensor.matmul(
                        ps[:, :], brow[brx:brx + 1, et * 128:(et + 1) * 128],
                        ones[0:1, 0:S], start=False, stop=True)
                dstl[et] = pool.tile([128, S], BF16, name="qkt", tag=tg)
                nc.vector.tensor_copy(dstl[et][:, :], ps[:, :])

            def emit_vaug(b):
                if b == 0:
                    WV.extend(wv_pool.tile_from(
                        d_wv[k * 128:(k + 1) * 128, :], name="wvt")
                        for k in range(KT_E))
                    WO.extend(wo_pool.tile_from(
                        d_wo[k * 128:(k + 1) * 128, :], name="wot")
                        for k in range(KT_E))
                # per 256-col head-pair block: [V_e(64) | ones(128) | V_o(64)]
                VA = VAb[b] = [None] * NT_B
                for tt in range(NT_B):
                    VA[tt] = va_pool.tile([128, HP * 256], BF16, name="vat",
                                          tag="va")
                    va3 = VA[tt].rearrange("p (blk c) -> p blk c", c=256)
                    nc.vector.memset(va3[:, :, 64:192], 1.0)
                    for ec, n in ((0, 512), (512, 256)):
                        ps = (p_mm.tile([128, 512], F32, name="vps", tag="mm")
                              if n == 512 else
                              p_mm.tile([128, 256], F32, name="vps2", tag="mm"))
                        for k in range(KT_E):
                            nc.tensor.matmul(
                                ps[:, :n], XT[(b, k)][:, tt * 128:(tt + 1) * 128],
                                WV[k][:, ec:ec + n],
                                start=(k == 0), stop=(k == KT_E - 1 and not use_bv))
                        if use_bv:
                            nc.tensor.matmul(
                                ps[:, :n], ones[0:1, 0:128],
                                brow[2:3, ec:ec + n], start=False, stop=True)
                        ps3 = ps.rearrange("p (h c) -> p h c", c=128)
                        nh_pairs = n // 128
                        blk0 = ec // 128
                        nc.vector.tensor_copy(
                            va3[:, blk0:blk0 + nh_pairs, 0:64],
                            ps3[:, 0:nh_pairs, 0:64])
                        nc.vector.tensor_copy(
                            va3[:, blk0:blk0 + nh_pairs, 192:256],
                            ps3[:, 0:nh_pairs, 64:128])

            def emit_scores(b, hp):
                QT, KTt = QTb[b], KTb[b]
                EXs = [None] * NT_B
                for jt in range(NT_B):
                    psc = p_sc.tile([128, 1024], F32, name="scps", tag="sc")
                    for hh in range(2):
                        o = hh * 64
                        nc.tensor.matmul(
                            psc[:, hh * 512:(hh + 1) * 512],
                            KTt[hp][o:o + 64, jt * 128:(jt + 1) * 128],
                            QT[hp][o:o + 64, :], start=True, stop=True)
                    ex = ex_pool.tile([128, 1024], BF16, name="ext", tag="ex")
                    mc = b * NT_B + jt
                    nc.scalar.activation(ex[:, :], psc[:, :], AF.Exp,
                                         bias=mcol[:, mc:mc + 1])
                    EXs[jt] = ex
                EXH[(b, hp)] = EXs

            def emit_av(b, hp):
                # P@V: [V|ones] -> att rows 0:64 + denom rows 64:128 (even)
                #      [ones|V] -> denom rows 0:64 + att rows 64:128 (odd)
                VA = VAb[b]
                EXs = EXH.pop((b, hp))
                psA = p_av.tile([128, 512], F32, name="avA", tag="av")
                psB = p_av.tile([128, 512], F32, name="avB", tag="av")
                for jt in range(NT_B):
                    nc.tensor.matmul(
                        psA[:, :], VA[jt][:, hp * 256:hp * 256 + 128],
                        EXs[jt][:, 0:512],
                        start=(jt == 0), stop=(jt == NT_B - 1))
                    nc.tensor.matmul(
                        psB[:, :], VA[jt][:, hp * 256 + 128:hp * 256 + 256],
                        EXs[jt][:, 512:1024],
                        start=(jt == 0), stop=(jt == NT_B - 1))
                # Release the PSUM banks fast: copy both banks to SBUF bf16
                # immediately; the normalize chain then runs from SBUF.
                uA = r_pool.tile([128, 512], BF16, name="uA", tag="uA")
                uB = r_pool.tile([128, 512], BF16, name="uB", tag="uB")
                nc.vector.tensor_copy(uA[:, :], psA[:, :])
                nc.vector.tensor_copy(uB[:, :], psB[:, :])
                # 1/s via ACT: ln(s) then exp(-x), BOTH before the broadcast
                # DMA so the Scalar queue never sits on a DMA wait.
                ra = r_pool.tile([128, 512], F32, name="ra", tag="ra")
                nc.scalar.activation(ra[64:128, :], uA[64:128, :], AF.Ln)
                nc.scalar.activation(ra[0:64, :], uB[0:64, :], AF.Ln)
                rp = r_pool.tile([128, 512], BF16, name="rp", tag="rp")
                nc.scalar.activation(rp[64:128, :], ra[64:128, :], AF.Exp,
                                     scale=-1.0)
                nc.scalar.activation(rp[0:64, :], ra[0:64, :], AF.Exp,
                                     scale=-1.0)
                rc = r_pool.tile([128, 512], BF16, name="rc", tag="rc")
                # DMA broadcast: one row -> 64 partitions (free-dim 0-stride
                # src; partition-stride-0 APs are rejected by lowering and
                # the gpsimd partition_broadcast ucode is broken on HW).
                nc.sync.dma_start(
                    bass.AP(rc.tensor, rc.offset,
                            [[512, 64], [1, 1], [1, 512]]),
                    bass.AP(rp.tensor, rp.offset + 64 * 512,
                            [[512, 1], [0, 64], [1, 512]]))
                nc.sync.dma_start(
                    bass.AP(rc.tensor, rc.offset + 64 * 512,
                            [[512, 64], [1, 1], [1, 512]]),
                    bass.AP(rp.tensor, rp.offset,
                            [[512, 1], [0, 64], [1, 512]]))
                att = att_pool.tile([128, S], BF16, name="attt", tag="attT")
                nc.vector.scalar_tensor_tensor(
                    att[0:64, :], uA[0:64, :], 1.0, rc[0:64, :],
                    op0=OP.mult, op1=OP.mult)
                nc.vector.scalar_tensor_tensor(
                    att[64:128, :], uB[64:128, :], 1.0, rc[64:128, :],
                    op0=OP.mult, op1=OP.mult)
                ATT[(b, hp)] = att

            # batch b's attention cycles carry batch b+1's Q/K projection
            # groups (emitted BEFORE each av so the PE FIFO has independent
            # work while the exp chain drains), and batch b+1's V_aug after
            # the last av (its V tiles only free then).
            emit_xt(0)
            for i in range(2 * KT_E):
                emit_qk_group(0, i)
            emit_vaug(0)
            for b in range(BL):
                if b + 1 < BL:
                    emit_xt(b + 1)
                emit_scores(b, 0)
                for hp in range(HP):
                    if hp + 1 < HP:
                        emit_scores(b, hp + 1)
                    if b + 1 < BL:
                        emit_qk_group(b + 1, 2 * hp)
                        emit_qk_group(b + 1, 2 * hp + 1)
                    emit_av(b, hp)
                if b + 1 < BL:
                    emit_vaug(b + 1)

        # ============ superphase B: O-proj, SelfOutput LN, FFN, LN ===========
        with ExitStack() as sb:
            b_pool = sb.enter_context(tc.tile_pool(name="b_consts", bufs=1))
            wso_pool = sb.enter_context(tc.tile_pool(name="wso", bufs=KT_E))
            wi_pool = sb.enter_context(tc.tile_pool(name="wi", bufs=KT_E))
            wout_pool = sb.enter_context(tc.tile_pool(name="wout", bufs=FT))
            h_pool = sb.enter_context(tc.tile_pool(name="h", bufs=2 * NT_B))
            ht_pool = sb.enter_context(tc.tile_pool(name="ht", bufs=2))
            fft_pool = sb.enter_context(tc.tile_pool(name="fft", bufs=FT))
            sq_pool = sb.enter_context(tc.tile_pool(name="sq", bufs=1))
            rs_pool = sb.enter_context(tc.tile_pool(name="rsd", bufs=3))
            out_pool = sb.enter_context(tc.tile_pool(name="outp", bufs=2, space="SBUF"))
            t_pool = sb.enter_context(tc.tile_pool(name="sb_s", bufs=4))
            p_mmb = sb.enter_context(tc.tile_pool(name="p_mmb", bufs=6,
                                                  space="PSUM"))

            bic = b_pool.tile_from(d_bic[:, :], name="bic") if use_bi else None
            WSO = [wso_pool.tile_from(d_wso[k * 128:(k + 1) * 128, :],
                                      name="wsot") for k in range(KT_E)]
            WI = [wi_pool.tile_from(d_wi[k * 128:(k + 1) * 128, :], name="wit")
                  for k in range(KT_E)]
            WOUT = [wout_pool.tile_from(d_wout[f * 128:(f + 1) * 128, :],
                                        name="woutt") for f in range(FT)]

            def layernorm(chunks, h_dst, gcol, use_g, use_bb, resid=None):
                """chunks: [(psum_ap, col0, n)]; h_dst: [128, E] fp32 out.
                resid: parallel list of sbuf fp32 APs added to psum first."""
                rtile = rs_pool.tile([128, E], BF16, name="rt", tag="rsd")
                if resid is not None:
                    for (ps, c0, n), rext in zip(chunks, resid):
                        nc.vector.scalar_tensor_tensor(
                            rtile[:, c0:c0 + n], ps, 1.0, rext,
                            op0=OP.mult, op1=OP.add)
                else:
                    # copy PSUM chunks out immediately so the GEMM psum bank
                    # is released after one DVE pass instead of being held
                    # through the whole layernorm chain
                    for (ps, c0, n) in chunks:
                        nc.vector.tensor_copy(rtile[:, c0:c0 + n], ps)
                srcs = [(rtile[:, c0:c0 + n], c0, n) for (_, c0, n) in chunks]
                s1 = t_pool.tile([128, 1], F32, name="s1", tag="s1")
                s1b = t_pool.tile([128, 1], F32, name="s1b", tag="s1b")
                nc.vector.reduce_sum(s1[:, :], srcs[0][0], axis=AX.X)
                nc.vector.reduce_sum(s1b[:, :], srcs[1][0], axis=AX.X)
                mu_n = t_pool.tile([128, 1], F32, name="mun", tag="mun")
                # mu_n = -(s1 + s1b)/E
                tmp = t_pool.tile([128, 1], F32, name="tmps", tag="tmps")
                nc.vector.scalar_tensor_tensor(
                    tmp[:, :], s1[:, :], 1.0, s1b[:, :], op0=OP.mult, op1=OP.add)
                nc.vector.tensor_scalar_mul(mu_n[:, :], tmp[:, :], -1.0 / E)
                ss = t_pool.tile([128, 1], F32, name="ssa", tag="ssa", bufs=16)
                ssb = t_pool.tile([128, 1], F32, name="ssb", tag="ssb", bufs=16)
                for (src, c0, n), acc in zip(srcs, (ss, ssb)):
                    sq = sq_pool.tile([128, 512], BF16, name="sqt", tag="sq")
                    nc.scalar.activation(sq[:, :n], src, AF.Square,
                                         accum_out=acc[:, :])
                # var = (ss+ssb)/E - mu^2 ; rstd = 1/sqrt(var + eps)
                musq = t_pool.tile([128, 1], F32, name="musq", tag="musq")
                nc.vector.scalar_tensor_tensor(
                    musq[:, :], mu_n[:, :], 1.0, mu_n[:, :],
                    op0=OP.mult, op1=OP.mult)
                veps = t_pool.tile([128, 1], F32, name="veps", tag="veps")
                nc.vector.scalar_tensor_tensor(
                    veps[:, :], ss[:, :], 1.0, ssb[:, :],
                    op0=OP.mult, op1=OP.add)
                veps2 = t_pool.tile([128, 1], F32, name="veps2", tag="veps2")
                nc.vector.tensor_scalar(
                    veps2[:, :], veps[:, :], 1.0 / E, EPS,
                    op0=OP.mult, op1=OP.add)
                veps3 = t_pool.tile([128, 1], F32, name="veps3", tag="veps3")
                nc.vector.scalar_tensor_tensor(
                    veps3[:, :], musq[:, :], -1.0, veps2[:, :],
                    op0=OP.mult, op1=OP.add)
                sd = t_pool.tile([128, 1], F32, name="sd", tag="sd")
                nc.scalar.sqrt(sd[:, :], veps3[:, :])
                rstd = t_pool.tile([128, 1], F32, name="rstd", tag="rstd")
                nc.vector.reciprocal(rstd[:, :], sd[:, :])
                for (src, c0, n) in srcs:
                    nc.vector.tensor_scalar(
                        h_dst[:, c0:c0 + n], src, mu_n[:, :], rstd[:, :],
                        op0=OP.add, op1=OP.mult)
                if use_g:
                    nc.vector.scalar_tensor_tensor(
                        h_dst[:, :], h_dst[:, :], 1.0,
                        gb[:, gcol * E:(gcol + 1) * E], op0=OP.mult, op1=OP.mult)
                if use_bb:
                    nc.vector.scalar_tensor_tensor(
                        h_dst[:, :], h_dst[:, :], 1.0,
                        gb[:, (gcol + 2) * E:(gcol + 3) * E],
                        op0=OP.mult, op1=OP.add)

            HHT, HTT, FFT, HTR = {}, {}, {}, {}

            def emit_oso(b):
                # ---- O-projection + residual -> xa (feature-major bf16) ----
                xa = [None] * KT_E
                for et in range(KT_E):
                    ps = p_mmb.tile([128, S], F32, name="ops", tag="mm")
                    for k in range(KT_E):
                        nc.tensor.matmul(
                            ps[:, :], WO[k][:, et * 128:(et + 1) * 128],
                            ATT[(b, k)][:, :],
                            start=(k == 0), stop=(k == KT_E - 1 and not use_bo))
                    if use_bo:
                        nc.tensor.matmul(
                            ps[:, :], brow[3:4, et * 128:(et + 1) * 128],
                            ones[0:1, 0:S], start=False, stop=True)
                    xa[et] = att_pool.tile([128, S], BF16, name="xat", tag="attT")
                    nc.vector.scalar_tensor_tensor(
                        xa[et][:, :], ps[:, :], 1.0, XT[(b, et)][:, :],
                        op0=OP.mult, op1=OP.add)

                # ---- SelfOutput GEMM + LN1 -> h (token-major fp32), hT ----
                hh_t = [None] * NT_B
                hT = ht_pool.tile([128, KT_E * S], BF16, name="htt", tag="ht")

                def emit_htrans(tt):
                    tps = [p_mmb.tile([128, 512], BF16, name="htp", tag="mm")
                           for _ in range(2)]
                    for et in range(KT_E):
                        sl = tps[et // 4][:, (et % 4) * 128:(et % 4 + 1) * 128]
                        nc.tensor.transpose(
                            sl, hh_t[tt][:, et * 128:(et + 1) * 128],
                            ident[:, :])
                    for et in range(KT_E):
                        sl = tps[et // 4][:, (et % 4) * 128:(et % 4 + 1) * 128]
                        nc.vector.tensor_copy(
                            hT[:, et * S + tt * 128:et * S + (tt + 1) * 128], sl)

                # skewed: h-transposes of tile tt are emitted after the
                # SO GEMM of tile tt+1, so the PE never waits on LN1
                for tt in range(NT_B):
                    ch = []
                    for ec, n in ((0, 512), (512, 256)):
                        ps = (p_mmb.tile([128, 512], F32, name="sops", tag="mm")
                              if n == 512 else
                              p_mmb.tile([128, 256], F32, name="sops2", tag="mm"))
                        for k in range(KT_E):
                            nc.tensor.matmul(
                                ps[:, :n], xa[k][:, tt * 128:(tt + 1) * 128],
                                WSO[k][:, ec:ec + n],
                                start=(k == 0),
                                stop=(k == KT_E - 1 and not use_bso))
                        if use_bso:
                            nc.tensor.matmul(
                                ps[:, :n], ones[0:1, 0:128],
                                brow[4:5, ec:ec + n], start=False, stop=True)
                        ch.append((ps[:, :n], ec, n))
                    hh_t[tt] = h_pool.tile([128, E], BF16, name="hht", tag="h")
                    layernorm(ch, hh_t[tt], 0, use_g1, use_b1)
                HHT[b], HTT[b] = hh_t, hT
                HTR[b] = emit_htrans

            def emit_wi(b):
                # ---- FFN Wi + GELU (full 512-token chunk: N=512 GEMMs) ----
                hT = HTT.pop(b)
                ffT = [None] * FT
                for ft in range(FT):
                    ps = p_mmb.tile([128, 512], F32, name="fips", tag="mm")
                    for k in range(KT_E):
                        nc.tensor.matmul(
                            ps[:, :], WI[k][:, ft * 128:(ft + 1) * 128],
                            hT[:, k * S:k * S + 512],
                            start=(k == 0), stop=(k == KT_E - 1))
                    ffT[ft] = fft_pool.tile([128, 512], BF16, name="fftt",
                                            tag="fft")
                    if use_bi:
                        nc.scalar.activation(ffT[ft][:, :], ps[:, :],
                                             AF.Gelu,
                                             bias=bic[:, ft:ft + 1])
                    else:
                        nc.scalar.activation(ffT[ft][:, :], ps[:, :],
                                             AF.Gelu)
                FFT[b] = ffT

            def emit_wout(b, inter=None):
                # ---- Wout + residual + LN2 -> out ----
                t0 = b * S
                ffT = FFT.pop(b)
                hh_t = HHT.pop(b)
                for tt in range(NT_B):
                    if inter is not None and tt >= 1:
                        inter(tt - 1)
                    ch = []
                    for ec, n in ((0, 512), (512, 256)):
                        ps = (p_mmb.tile([128, 512], F32, name="wops",
                                         tag="mm") if n == 512 else
                              p_mmb.tile([128, 256], F32, name="wops2",
                                         tag="mm"))
                        for f in range(FT):
                            nc.tensor.matmul(
                                ps[:, :n],
                                ffT[f][:, tt * 128:(tt + 1) * 128],
                                WOUT[f][:, ec:ec + n],
                                start=(f == 0),
                                stop=(f == FT - 1 and not use_bout))
                        if use_bout:
                            nc.tensor.matmul(
                                ps[:, :n], ones[0:1, 0:128],
                                brow[5:6, ec:ec + n], start=False,
                                stop=True)
                        ch.append((ps[:, :n], ec, n))
                    otile = out_pool.tile([128, E], F32, name="ot",
                                          tag="outp")
                    resid = [hh_t[tt][:, ec:ec + n] for (_, ec, n) in ch]
                    layernorm(ch, otile, 1, use_g2, use_b2, resid=resid)
                    nc.gpsimd.dma_start(
                        d_out[t0 + tt * 128:t0 + (tt + 1) * 128, :],
                        otile[:, :])

            # software-pipelined: O/SO/LN1 of batch b+1 are emitted between
            # Wi(b) and Wout(b); the h-transposes of b+1 are interleaved
            # into Wout(b)'s token-tile loop so they land after their LN1
            # chains have completed, and the independent Wout work is never
            # queued behind an LN-stalled transpose.
            emit_oso(0)
            for tt in range(NT_B):
                HTR[0](tt)
            for b in range(BL):
                emit_wi(b)
                if b + 1 < BL:
                    emit_oso(b + 1)
                emit_wout(b)
                if b + 1 < BL:
                    for tt in range(NT_B):
                        HTR[b + 1](tt)
                    del HTR[b + 1]
    nc.compile()
    return nc


def _get_program(flags):
    key = ("prog", flags)
    if key not in _CACHE:
        _CACHE[key] = _build(flags)
    return _CACHE[key]


def kernel(x, mask, Wq, bq, Wk, bk, Wv, bv, Wo, bo,
           Wso, bso, gso, beso, Wi, bi, Wout, bout, gout, beout):
    from concourse.bass_utils import run_bass_kernel_spmd

    x = np.asarray(x, np.float32)
    mask = np.asarray(mask)
    sc = 1.0 / float(np.sqrt(np.float32(DK)))

    z = lambda a: not np.any(np.asarray(a))
    one = lambda a: bool(np.all(np.asarray(a) == 1.0))
    flags = (not z(bq), not z(bk), not z(bv), not z(bo), not z(bso),
             not z(bi), not z(bout),
             not one(gso), not z(beso), not one(gout), not z(beout))
    nc = _get_program(flags)

    wq_b = _bf(np.asarray(Wq, np.float32) * sc)
    wk_b, wv_b, wo_b, wso_b = _bf(Wk), _bf(Wv), _bf(Wo), _bf(Wso)
    wi_b, wout_b = _bf(Wi), _bf(Wout)
    identb = _bf(np.eye(128))
    onesr = _bf(np.ones((1, 512)))

    brow = np.zeros((7, FF), np.float32)
    brow[0, :E] = np.asarray(bq, np.float32) * sc
    for i, v in enumerate((bk, bv, bo, bso, bout)):
        brow[i + 1, :E] = v
    brow[6, :] = bi
    brow = _bf(brow)
    bicol = np.asarray(bi, np.float32).reshape(FF // 128, 128).T.copy()
    gbt = np.zeros((128, 4 * E), np.float32)
    for i, g in enumerate((gso, gout, beso, beout)):   # gamma1|gamma2|beta1|beta2
        gbt[:, i * E:(i + 1) * E] = np.broadcast_to(
            np.asarray(g, np.float32).reshape(1, E), (128, E))

    in_maps = []
    for c in range(NCORES):
        xs = np.ascontiguousarray(x[c * BL:(c + 1) * BL].reshape(T, E).T)
        ms = np.asarray(mask[c * BL:(c + 1) * BL]).reshape(BL, S)
        mb = np.where(ms == 0, np.float32(MASK_NEG), np.float32(0.0))
        # mcol[p, b*4+jt] = mask bias of kv position jt*128+p in batch b
        mcolv = np.ascontiguousarray(
            mb.reshape(BL, S // 128, 128).transpose(2, 0, 1).reshape(
                128, BL * (S // 128)).astype(np.float32))
        in_maps.append({
            "x": _bf(xs), "wq": wq_b, "wk": wk_b, "wv": wv_b, "wo": wo_b,
            "wso": wso_b, "wi": wi_b, "wout": wout_b, "mcol": mcolv,
            "ident": identb, "onesrow": onesr,
            "brow": brow, "bicol": bicol, "gb": gbt,
        })

    trace = os.environ.get("KERNEL_TRACE", "0") == "1"
    res = run_bass_kernel_spmd(nc, in_maps, core_ids=list(range(NCORES)),
                               trace=trace)
    if trace and res.exec_time_ns is not None:
        print(f"HW exec time: {res.exec_time_ns} ns")
        if res.instructions_and_trace is not None:
            print(f"trace: {res.instructions_and_trace[1]}")
    out = np.concatenate([r["out"].reshape(BL, S, E) for r in res.results],
                         axis=0)
    return np.ascontiguousarray(out.astype(np.float32))


# revision 38
# speedup vs baseline: 1.2015x; 1.0053x over previous
"""BERT-base encoder layer on 8 Trainium2 NeuronCores (Bass/Tile).

Sharding: data-parallel over batch. Full inputs [32, 512, 768] split into 8
shards of 4 batches (2048 tokens); every core runs the same NEFF on its shard
(SPMD, no collectives); host concatenates the outputs.

All GEMMs run on the PE in bf16 with fp32 PSUM accumulation; softmax and
layernorm statistics run in fp32. 1/sqrt(dk) is folded into Wq on the host.

Attention is computed in TRANSPOSED score layout: scoresT[kv, q] = K^T Q with
kv on partitions. This makes the additive attention mask a per-partition bias
applied for free inside the exp activation, and it removes the PE transpose
of the probabilities entirely (exp(scoresT) feeds P@V directly as the moving
operand). Softmax denominators come free out of the P@V matmul by augmenting
V with 64 columns of ones ([V|ones] / [ones|V] per head pair): the same
matmul that produces the 64 attention rows produces 64 broadcast rows of the
denominator in the other half of the PSUM bank. The reciprocal runs on the
Scalar engine as exp(-ln(s)) (the DVE reciprocal is a slow multi-pass op),
a stride-0-free-dim DMA moves each head's 1/s row into the partition range
of its attention rows, and a DVE multiply applies the normalization.

Phase B (O-proj, LN1, FFN, LN2) is software-pipelined across batches so the
PE always has independent matmul work during GELU/layernorm tails, and all
GEMM PSUM banks are released after a single copy/STT pass.
"""

import os
import numpy as np
import ml_dtypes

B, S, E, H, DK, FF = 32, 512, 768, 12, 64, 3072
NCORES = 8
BL = B // NCORES          # batches per core = 4
T = BL * S                # tokens per core = 2048
EPS = 1e-12
MASK_NEG = -87.0          # stays inside exp-table range; exp() == 0 in fp32

_CACHE = {}


def _bf(a):
    return np.ascontiguousarray(np.asarray(a, np.float32).astype(ml_dtypes.bfloat16))


def _build(flags):
    import concourse.bass as bass
    import concourse.bacc as bacc
    import concourse.mybir as mybir
    import concourse.tile as tile
    from contextlib import ExitStack

    (use_bq, use_bk, use_bv, use_bo, use_bso, use_bi, use_bout,
     use_g1, use_b1, use_g2, use_b2) = flags

    AF = mybir.ActivationFunctionType
    OP = mybir.AluOpType
    AX = mybir.AxisListType
    BF16 = mybir.dt.bfloat16
    F32 = mybir.dt.float32

    nc = bacc.Bacc("TRN2", target_bir_lowering=False)

    # Both Exp and Ln are used in the attention inner loop. The default
    # act-table chooser assigns them different table sets, forcing a ~1.3us
    # ACT_TABLE_LOAD on every switch. Steer the chooser to the one set that
    # holds both (natural_log_exp_and_others) by hiding Exp/Ln in the other
    # sets this instance sees; set ids keep their act_info.json positions.
    import types as _types
    from concourse.hw_specs import get_activation_tables as _gat

    def _patched_act_loads(self):
        import bass_rust as _br
        has_act = any(isinstance(i, mybir.InstActivation)
                      for blk in self.main_func.blocks
                      for i in blk.instructions)
        if not has_act:
            return
        AFT = mybir.ActivationFunctionType
        items = []
        for name, funcs in _gat(self.m.arch).items():
            if name != "natural_log_exp_and_others":
                funcs = {f for f in funcs if f not in (AFT.Exp, AFT.Ln)}
            items.append((name, funcs))
        _br.insert_act_table_loads(self, items)

    nc.insert_act_table_loads = _types.MethodType(_patched_act_loads, nc)

    d_x = nc.dram_tensor("x", (E, T), BF16, kind="ExternalInput")
    d_wq = nc.dram_tensor("wq", (E, E), BF16, kind="ExternalInput")
    d_wk = nc.dram_tensor("wk", (E, E), BF16, kind="ExternalInput")
    d_wv = nc.dram_tensor("wv", (E, E), BF16, kind="ExternalInput")
    d_wo = nc.dram_tensor("wo", (E, E), BF16, kind="ExternalInput")
    d_wso = nc.dram_tensor("wso", (E, E), BF16, kind="ExternalInput")
    d_wi = nc.dram_tensor("wi", (E, FF), BF16, kind="ExternalInput")
    d_wout = nc.dram_tensor("wout", (FF, E), BF16, kind="ExternalInput")
    # per-(batch, kv-tile) additive mask bias column: [128, BL*NT_B] fp32
    d_mcol = nc.dram_tensor("mcol", (128, BL * (S // 128)), mybir.dt.float32,
                            kind="ExternalInput")
    d_id = nc.dram_tensor("ident", (128, 128), BF16, kind="ExternalInput")
    d_ones = nc.dram_tensor("onesrow", (1, 512), BF16, kind="ExternalInput")
    # bias rows: 0=bq/8, 1=bk, 2=bv, 3=bo, 4=bso, 5=bout, 6=bi (full FF width)
    d_brow = nc.dram_tensor("brow", (7, FF), BF16, kind="ExternalInput")
    d_bic = nc.dram_tensor("bicol", (128, FF // 128), F32, kind="ExternalInput")
    # gamma1 | beta1 | gamma2 | beta2, each [128, 768] partition-broadcast
    d_gb = nc.dram_tensor("gb", (128, 4 * E), F32, kind="ExternalInput")
    d_out = nc.dram_tensor("out", (T, E), F32, kind="ExternalOutput")

    KT_E = E // 128    # 6
    NT_B = S // 128    # 4
    FT = FF // 128     # 24
    HP = H // 2        # 6

    need_gb = use_g1 or use_b1 or use_g2 or use_b2
    need_brow = use_bq or use_bk or use_bv or use_bo or use_bso or use_bout

    with ExitStack() as ctx:
        tc = ctx.enter_context(tile.TileContext(nc))

        p_mm = ctx.enter_context(tc.tile_pool(name="p_mm", bufs=2, space="PSUM"))

        c_pool = ctx.enter_context(tc.tile_pool(name="consts", bufs=1))
        xt_pool = ctx.enter_context(tc.tile_pool(name="xt", bufs=BL * KT_E))
        att_pool = ctx.enter_context(tc.tile_pool(name="attp", bufs=BL * KT_E + 4))

        wo_pool = ctx.enter_context(tc.tile_pool(name="wo", bufs=KT_E))
        WO = []
        ident = c_pool.tile_from(d_id[:, :], name="ident")
        ones = c_pool.tile_from(d_ones[:, :], name="ones")
        brow = c_pool.tile_from(d_brow[:, :], name="brow") if need_brow else None
        gb = c_pool.tile_from(d_gb[:, :], name="gb") if need_gb else None

        XT = {}    # (b, kt) -> [128, S] bf16, feature-major x
        ATT = {}   # (b, kt) -> [128, S] bf16, feature-major normalized attention

        # ================= superphase A: x^T, QKV, attention =================
        with ExitStack() as sa:
            a_pool = sa.enter_context(tc.tile_pool(name="a_consts", bufs=1))
            wq_pool = sa.enter_context(tc.tile_pool(name="wq", bufs=KT_E))
            wk_pool = sa.enter_context(tc.tile_pool(name="wk", bufs=KT_E))
            wv_pool = sa.enter_context(tc.tile_pool(name="wv", bufs=KT_E))
            qt_pool = sa.enter_context(tc.tile_pool(name="qt", bufs=KT_E + 3))
            kt_pool = sa.enter_context(tc.tile_pool(name="kt", bufs=KT_E + 3))
            va_pool = sa.enter_context(tc.tile_pool(name="va", bufs=NT_B + 1))
            # scoresT psum: [128, 1024] fp32 = 2 banks, both hh halves
            p_sc = sa.enter_context(tc.tile_pool(name="p_sc", bufs=2,
                                                 space="PSUM"))
            p_av = sa.enter_context(tc.tile_pool(name="p_av", bufs=2,
                                                 space="PSUM"))
            ex_pool = sa.enter_context(tc.tile_pool(name="ex", bufs=7))
            r_pool = sa.enter_context(tc.tile_pool(name="rp", bufs=3))

            mcol = a_pool.tile_from(d_mcol[:, :], name="mcol")
            WQ = [wq_pool.tile_from(d_wq[k * 128:(k + 1) * 128, :], name="wqt")
                  for k in range(KT_E)]
            WK = [wk_pool.tile_from(d_wk[k * 128:(k + 1) * 128, :], name="wkt")
                  for k in range(KT_E)]
            WV = []

            QTb, KTb, VAb, EXH = {}, {}, {}, {}

            def emit_xt(b):
                t0 = b * S
                for kt in range(KT_E):
                    XT[(b, kt)] = xt_pool.tile([128, S], BF16, name="xtt",
                                               tag="xt")
                    nc.gpsimd.dma_start(
                        XT[(b, kt)][:, :],
                        d_x[kt * 128:(kt + 1) * 128, t0:t0 + S])

            def emit_qk_group(b, idx):
                # idx 0..5: Q projection tile et=idx; 6..11: K tile et=idx-6
                if b not in QTb:
                    QTb[b], KTb[b] = [None] * KT_E, [None] * KT_E
                if idx < KT_E:
                    Wt, dstl, pool, ub, brx, tg = WQ, QTb[b], qt_pool, use_bq, 0, "qt"
                    et = idx
                else:
                    Wt, dstl, pool, ub, brx, tg = WK, KTb[b], kt_pool, use_bk, 1, "kt"
                    et = idx - KT_E
                ps = p_mm.tile([128, S], F32, name="qkps", tag="mm")
                for k in range(KT_E):
                    nc.tensor.matmul(
                        ps[:, :], Wt[k][:, et * 128:(et + 1) * 128],
                        XT[(b, k)][:, :],
                        start=(k == 0), stop=(k == KT_E - 1 and not ub))
                if ub:
                    nc.tensor.matmul(
                        ps[:, :], brow[brx:brx + 1, et * 128:(et + 1) * 128],
                        ones[0:1, 0:S], start=False, stop=True)
                dstl[et] = pool.tile([128, S], BF16, name="qkt", tag=tg)
                nc.vector.tensor_copy(dstl[et][:, :], ps[:, :])

            def emit_vaug(b):
                if b == 0:
                    WV.extend(wv_pool.tile_from(
                        d_wv[k * 128:(k + 1) * 128, :], name="wvt")
                        for k in range(KT_E))
                    WO.extend(wo_pool.tile_from(
                        d_wo[k * 128:(k + 1) * 128, :], name="wot")
                        for k in range(KT_E))
                # per 256-col head-pair block: [V_e(64) | ones(128) | V_o(64)]
                VA = VAb[b] = [None] * NT_B
                for tt in range(NT_B):
                    VA[tt] = va_pool.tile([128, HP * 256], BF16, name="vat",
                                          tag="va")
                    va3 = VA[tt].rearrange("p (blk c) -> p blk c", c=256)
                    nc.vector.memset(va3[:, :, 64:192], 1.0)
                    for ec, n in ((0, 512), (512, 256)):
                        ps = (p_mm.tile([128, 512], F32, name="vps", tag="mm")
                              if n == 512 else
                              p_mm.tile([128, 256], F32, name="vps2", tag="mm"))
                        for k in range(KT_E):
                            nc.tensor.matmul(
                                ps[:, :n], XT[(b, k)][:, tt * 128:(tt + 1) * 128],
                                WV[k][:, ec:ec + n],
                                start=(k == 0), stop=(k == KT_E - 1 and not use_bv))
                        if use_bv:
                            nc.tensor.matmul(
                                ps[:, :n], ones[0:1, 0:128],
                                brow[2:3, ec:ec + n], start=False, stop=True)
                        ps3 = ps.rearrange("p (h c) -> p h c", c=128)
                        nh_pairs = n // 128
                        blk0 = ec // 128
                        nc.vector.tensor_copy(
                            va3[:, blk0:blk0 + nh_pairs, 0:64],
                            ps3[:, 0:nh_pairs, 0:64])
                        nc.vector.tensor_copy(
                            va3[:, blk0:blk0 + nh_pairs, 192:256],
                            ps3[:, 0:nh_pairs, 64:128])

            def emit_scores(b, hp):
                QT, KTt = QTb[b], KTb[b]
                EXs = [None] * NT_B
                for jt in range(NT_B):
                    psc = p_sc.tile([128, 1024], F32, name="scps", tag="sc")
                    for hh in range(2):
                        o = hh * 64
                        nc.tensor.matmul(
                            psc[:, hh * 512:(hh + 1) * 512],
                            KTt[hp][o:o + 64, jt * 128:(jt + 1) * 128],
                            QT[hp][o:o + 64, :], start=True, stop=True)
                    ex = ex_pool.tile([128, 1024], BF16, name="ext", tag="ex")
                    mc = b * NT_B + jt
                    nc.scalar.activation(ex[:, :], psc[:, :], AF.Exp,
                                         bias=mcol[:, mc:mc + 1])
                    EXs[jt] = ex
                EXH[(b, hp)] = EXs

            def emit_av(b, hp):
                # P@V: [V|ones] -> att rows 0:64 + denom rows 64:128 (even)
                #      [ones|V] -> denom rows 0:64 + att rows 64:128 (odd)
                VA = VAb[b]
                EXs = EXH.pop((b, hp))
                psA = p_av.tile([128, 512], F32, name="avA", tag="av")
                psB = p_av.tile([128, 512], F32, name="avB", tag="av")
                for jt in range(NT_B):
                    nc.tensor.matmul(
                        psA[:, :], VA[jt][:, hp * 256:hp * 256 + 128],
                        EXs[jt][:, 0:512],
                        start=(jt == 0), stop=(jt == NT_B - 1))
                    nc.tensor.matmul(
                        psB[:, :], VA[jt][:, hp * 256 + 128:hp * 256 + 256],
                        EXs[jt][:, 512:1024],
                        start=(jt == 0), stop=(jt == NT_B - 1))
                # Release the PSUM banks fast: copy both banks to SBUF bf16
                # immediately; the normalize chain then runs from SBUF.
                uA = r_pool.tile([128, 512], BF16, name="uA", tag="uA")
                uB = r_pool.tile([128, 512], BF16, name="uB", tag="uB")
                nc.vector.tensor_copy(uA[:, :], psA[:, :])
                nc.vector.tensor_copy(uB[:, :], psB[:, :])
                # 1/s via ACT: ln(s) then exp(-x), BOTH before the broadcast
                # DMA so the Scalar queue never sits on a DMA wait.
                ra = r_pool.tile([128, 512], F32, name="ra", tag="ra")
                nc.scalar.activation(ra[64:128, :], uA[64:128, :], AF.Ln)
                nc.scalar.activation(ra[0:64, :], uB[0:64, :], AF.Ln)
                rp = r_pool.tile([128, 512], BF16, name="rp", tag="rp")
                nc.scalar.activation(rp[64:128, :], ra[64:128, :], AF.Exp,
                                     scale=-1.0)
                nc.scalar.activation(rp[0:64, :], ra[0:64, :], AF.Exp,
                                     scale=-1.0)
                rc = r_pool.tile([128, 512], BF16, name="rc", tag="rc")
                # DMA broadcast: one row -> 64 partitions (free-dim 0-stride
                # src; partition-stride-0 APs are rejected by lowering and
                # the gpsimd partition_broadcast ucode is broken on HW).
                nc.sync.dma_start(
                    bass.AP(rc.tensor, rc.offset,
                            [[512, 64], [1, 1], [1, 512]]),
                    bass.AP(rp.tensor, rp.offset + 64 * 512,
                            [[512, 1], [0, 64], [1, 512]]))
                nc.sync.dma_start(
                    bass.AP(rc.tensor, rc.offset + 64 * 512,
                            [[512, 64], [1, 1], [1, 512]]),
                    bass.AP(rp.tensor, rp.offset,
                            [[512, 1], [0, 64], [1, 512]]))
                att = att_pool.tile([128, S], BF16, name="attt", tag="attT")
                nc.vector.scalar_tensor_tensor(
                    att[0:64, :], uA[0:64, :], 1.0, rc[0:64, :],
                    op0=OP.mult, op1=OP.mult)
                nc.vector.scalar_tensor_tensor(
                    att[64:128, :], uB[64:128, :], 1.0, rc[64:128, :],
                    op0=OP.mult, op1=OP.mult)
                ATT[(b, hp)] = att

            # batch b's attention cycles carry batch b+1's Q/K projection
            # groups (emitted BEFORE each av so the PE FIFO has independent
            # work while the exp chain drains), and batch b+1's V_aug after
            # the last av (its V tiles only free then).
            emit_xt(0)
            for i in range(2 * KT_E):
                emit_qk_group(0, i)
            emit_vaug(0)
            for b in range(BL):
                if b + 1 < BL:
                    emit_xt(b + 1)
                emit_scores(b, 0)
                for hp in range(HP):
                    if hp + 1 < HP:
                        emit_scores(b, hp + 1)
                    if b + 1 < BL:
                        emit_qk_group(b + 1, 2 * hp)
                        emit_qk_group(b + 1, 2 * hp + 1)
                    emit_av(b, hp)
                if b + 1 < BL:
                    emit_vaug(b + 1)

        # ============ superphase B: O-proj, SelfOutput LN, FFN, LN ===========
        with ExitStack() as sb:
            b_pool = sb.enter_context(tc.tile_pool(name="b_consts", bufs=1))
            wso_pool = sb.enter_context(tc.tile_pool(name="wso", bufs=KT_E))
            wi_pool = sb.enter_context(tc.tile_pool(name="wi", bufs=KT_E))
            wout_pool = sb.enter_context(tc.tile_pool(name="wout", bufs=FT))
            h_pool = sb.enter_context(tc.tile_pool(name="h", bufs=2 * NT_B))
            ht_pool = sb.enter_context(tc.tile_pool(name="ht", bufs=2))
            fft_pool = sb.enter_context(tc.tile_pool(name="fft", bufs=FT))
            sq_pool = sb.enter_context(tc.tile_pool(name="sq", bufs=1))
            rs_pool = sb.enter_context(tc.tile_pool(name="rsd", bufs=3))
            out_pool = sb.enter_context(tc.tile_pool(name="outp", bufs=2, space="SBUF"))
            t_pool = sb.enter_context(tc.tile_pool(name="sb_s", bufs=4))
            p_mmb = sb.enter_context(tc.tile_pool(name="p_mmb", bufs=6,
                                                  space="PSUM"))

            bic = b_pool.tile_from(d_bic[:, :], name="bic") if use_bi else None
            WSO = [wso_pool.tile_from(d_wso[k * 128:(k + 1) * 128, :],
                                      name="wsot") for k in range(KT_E)]
            WI = [wi_pool.tile_from(d_wi[k * 128:(k + 1) * 128, :], name="wit")
                  for k in range(KT_E)]
            WOUT = [wout_pool.tile_from(d_wout[f * 128:(f + 1) * 128, :],
                                        name="woutt") for f in range(FT)]

            def layernorm(chunks, h_dst, gcol, use_g, use_bb, resid=None):
                """chunks: [(psum_ap, col0, n)]; h_dst: [128, E] fp32 out.
                resid: parallel list of sbuf fp32 APs added to psum first."""
                rtile = rs_pool.tile([128, E], BF16, name="rt", tag="rsd")
                s1 = t_pool.tile([128, 1], F32, name="s1", tag="s1")
                s1b = t_pool.tile([128, 1], F32, name="s1b", tag="s1b")
                # the copy/residual STT also produces the row-sums via
                # accum_out, saving a separate reduce pass per chunk
                if resid is not None:
                    for (ps, c0, n), rext, acc in zip(chunks, resid, (s1, s1b)):
                        nc.vector.scalar_tensor_tensor(
                            rtile[:, c0:c0 + n], ps, 1.0, rext,
                            op0=OP.mult, op1=OP.add, accum_out=acc[:, :])
                else:
                    # in1 is read but discarded by the bypass op; any SBUF
                    # tile of the right shape works (PSUM+PSUM is illegal)
                    for (ps, c0, n), acc in zip(chunks, (s1, s1b)):
                        nc.vector.scalar_tensor_tensor(
                            rtile[:, c0:c0 + n], ps, 1.0,
                            XT[(0, 0)][:, 0:n],
                            op0=OP.mult, op1=OP.bypass, accum_out=acc[:, :])
                srcs = [(rtile[:, c0:c0 + n], c0, n) for (_, c0, n) in chunks]
                mu_n = t_pool.tile([128, 1], F32, name="mun", tag="mun")
                # mu_n = -(s1 + s1b)/E
                tmp = t_pool.tile([128, 1], F32, name="tmps", tag="tmps")
                nc.vector.scalar_tensor_tensor(
                    tmp[:, :], s1[:, :], 1.0, s1b[:, :], op0=OP.mult, op1=OP.add)
                nc.vector.tensor_scalar_mul(mu_n[:, :], tmp[:, :], -1.0 / E)
                ss = t_pool.tile([128, 1], F32, name="ssa", tag="ssa", bufs=16)
                ssb = t_pool.tile([128, 1], F32, name="ssb", tag="ssb", bufs=16)
                for (src, c0, n), acc in zip(srcs, (ss, ssb)):
                    sq = sq_pool.tile([128, 512], BF16, name="sqt", tag="sq")
                    nc.scalar.activation(sq[:, :n], src, AF.Square,
                                         accum_out=acc[:, :])
                # var = (ss+ssb)/E - mu^2 ; rstd = 1/sqrt(var + eps)
                musq = t_pool.tile([128, 1], F32, name="musq", tag="musq")
                nc.vector.scalar_tensor_tensor(
                    musq[:, :], mu_n[:, :], 1.0, mu_n[:, :],
                    op0=OP.mult, op1=OP.mult)
                veps = t_pool.tile([128, 1], F32, name="veps", tag="veps")
                nc.vector.scalar_tensor_tensor(
                    veps[:, :], ss[:, :], 1.0, ssb[:, :],
                    op0=OP.mult, op1=OP.add)
                veps2 = t_pool.tile([128, 1], F32, name="veps2", tag="veps2")
                nc.vector.tensor_scalar(
                    veps2[:, :], veps[:, :], 1.0 / E, EPS,
                    op0=OP.mult, op1=OP.add)
                veps3 = t_pool.tile([128, 1], F32, name="veps3", tag="veps3")
                nc.vector.scalar_tensor_tensor(
                    veps3[:, :], musq[:, :], -1.0, veps2[:, :],
                    op0=OP.mult, op1=OP.add)
                sd = t_pool.tile([128, 1], F32, name="sd", tag="sd")
                nc.scalar.sqrt(sd[:, :], veps3[:, :])
                rstd = t_pool.tile([128, 1], F32, name="rstd", tag="rstd")
                nc.vector.reciprocal(rstd[:, :], sd[:, :])
                for (src, c0, n) in srcs:
                    nc.vector.tensor_scalar(
                        h_dst[:, c0:c0 + n], src, mu_n[:, :], rstd[:, :],
                        op0=OP.add, op1=OP.mult)
                if use_g:
                    nc.vector.scalar_tensor_tensor(
                        h_dst[:, :], h_dst[:, :], 1.0,
                        gb[:, gcol * E:(gcol + 1) * E], op0=OP.mult, op1=OP.mult)
                if use_bb:
                    nc.vector.scalar_tensor_tensor(
                        h_dst[:, :], h_dst[:, :], 1.0,
                        gb[:, (gcol + 2) * E:(gcol + 3) * E],
                        op0=OP.mult, op1=OP.add)

            HHT, HTT, FFT, HTR = {}, {}, {}, {}

            def emit_oso(b):
                # ---- O-projection + residual -> xa (feature-major bf16) ----
                xa = [None] * KT_E
                for et in range(KT_E):
                    ps = p_mmb.tile([128, S], F32, name="ops", tag="mm")
                    for k in range(KT_E):
                        nc.tensor.matmul(
                            ps[:, :], WO[k][:, et * 128:(et + 1) * 128],
                            ATT[(b, k)][:, :],
                            start=(k == 0), stop=(k == KT_E - 1 and not use_bo))
                    if use_bo:
                        nc.tensor.matmul(
                            ps[:, :], brow[3:4, et * 128:(et + 1) * 128],
                            ones[0:1, 0:S], start=False, stop=True)
                    xa[et] = att_pool.tile([128, S], BF16, name="xat", tag="attT")
                    nc.vector.scalar_tensor_tensor(
                        xa[et][:, :], ps[:, :], 1.0, XT[(b, et)][:, :],
                        op0=OP.mult, op1=OP.add)

                # ---- SelfOutput GEMM + LN1 -> h (token-major fp32), hT ----
                hh_t = [None] * NT_B
                hT = ht_pool.tile([128, KT_E * S], BF16, name="htt", tag="ht")

                def emit_htrans(tt):
                    tps = [p_mmb.tile([128, 512], BF16, name="htp", tag="mm")
                           for _ in range(2)]
                    for et in range(KT_E):
                        sl = tps[et // 4][:, (et % 4) * 128:(et % 4 + 1) * 128]
                        nc.tensor.transpose(
                            sl, hh_t[tt][:, et * 128:(et + 1) * 128],
                            ident[:, :])
                    for et in range(KT_E):
                        sl = tps[et // 4][:, (et % 4) * 128:(et % 4 + 1) * 128]
                        nc.vector.tensor_copy(
                            hT[:, et * S + tt * 128:et * S + (tt + 1) * 128], sl)

                # skewed: h-transposes of tile tt are emitted after the
                # SO GEMM of tile tt+1, so the PE never waits on LN1
                for tt in range(NT_B):
                    ch = []
                    for ec, n in ((0, 512), (512, 256)):
                        ps = (p_mmb.tile([128, 512], F32, name="sops", tag="mm")
                              if n == 512 else
                              p_mmb.tile([128, 256], F32, name="sops2", tag="mm"))
                        for k in range(KT_E):
                            nc.tensor.matmul(
                                ps[:, :n], xa[k][:, tt * 128:(tt + 1) * 128],
                                WSO[k][:, ec:ec + n],
                                start=(k == 0),
                                stop=(k == KT_E - 1 and not use_bso))
                        if use_bso:
                            nc.tensor.matmul(
                                ps[:, :n], ones[0:1, 0:128],
                                brow[4:5, ec:ec + n], start=False, stop=True)
                        ch.append((ps[:, :n], ec, n))
                    hh_t[tt] = h_pool.tile([128, E], BF16, name="hht", tag="h")
                    layernorm(ch, hh_t[tt], 0, use_g1, use_b1)
                HHT[b], HTT[b] = hh_t, hT
                HTR[b] = emit_htrans

            def emit_wi(b):
                # ---- FFN Wi + GELU (full 512-token chunk: N=512 GEMMs) ----
                hT = HTT.pop(b)
                ffT = [None] * FT
                for ft in range(FT):
                    ps = p_mmb.tile([128, 512], F32, name="fips", tag="mm")
                    for k in range(KT_E):
                        nc.tensor.matmul(
                            ps[:, :], WI[k][:, ft * 128:(ft + 1) * 128],
                            hT[:, k * S:k * S + 512],
                            start=(k == 0), stop=(k == KT_E - 1))
                    ffT[ft] = fft_pool.tile([128, 512], BF16, name="fftt",
                                            tag="fft")
                    if use_bi:
                        nc.scalar.activation(ffT[ft][:, :], ps[:, :],
                                             AF.Gelu,
                                             bias=bic[:, ft:ft + 1])
                    else:
                        nc.scalar.activation(ffT[ft][:, :], ps[:, :],
                                             AF.Gelu)
                FFT[b] = ffT

            def emit_wout(b, inter=None):
                # ---- Wout + residual + LN2 -> out ----
                t0 = b * S
                ffT = FFT.pop(b)
                hh_t = HHT.pop(b)
                for tt in range(NT_B):
                    if inter is not None and tt >= 1:
                        inter(tt - 1)
                    ch = []
                    for ec, n in ((0, 512), (512, 256)):
                        ps = (p_mmb.tile([128, 512], F32, name="wops",
                                         tag="mm") if n == 512 else
                              p_mmb.tile([128, 256], F32, name="wops2",
                                         tag="mm"))
                        for f in range(FT):
                            nc.tensor.matmul(
                                ps[:, :n],
                                ffT[f][:, tt * 128:(tt + 1) * 128],
                                WOUT[f][:, ec:ec + n],
                                start=(f == 0),
                                stop=(f == FT - 1 and not use_bout))
                        if use_bout:
                            nc.tensor.matmul(
                                ps[:, :n], ones[0:1, 0:128],
                                brow[5:6, ec:ec + n], start=False,
                                stop=True)
                        ch.append((ps[:, :n], ec, n))
                    otile = out_pool.tile([128, E], F32, name="ot",
                                          tag="outp")
                    resid = [hh_t[tt][:, ec:ec + n] for (_, ec, n) in ch]
                    layernorm(ch, otile, 1, use_g2, use_b2, resid=resid)
                    nc.gpsimd.dma_start(
                        d_out[t0 + tt * 128:t0 + (tt + 1) * 128, :],
                        otile[:, :])

            # software-pipelined: O/SO/LN1 of batch b+1 are emitted between
            # Wi(b) and Wout(b); the h-transposes of b+1 are interleaved
            # into Wout(b)'s token-tile loop so they land after their LN1
            # chains have completed, and the independent Wout work is never
            # queued behind an LN-stalled transpose.
            emit_oso(0)
            for tt in range(NT_B):
                HTR[0](tt)
            for b in range(BL):
                emit_wi(b)
                if b + 1 < BL:
                    emit_oso(b + 1)
                emit_wout(b)
                if b + 1 < BL:
                    for tt in range(NT_B):
                        HTR[b + 1](tt)
                    del HTR[b + 1]
    nc.compile()
    return nc


def _get_program(flags):
    key = ("prog", flags)
    if key not in _CACHE:
        _CACHE[key] = _build(flags)
    return _CACHE[key]


def kernel(x, mask, Wq, bq, Wk, bk, Wv, bv, Wo, bo,
           Wso, bso, gso, beso, Wi, bi, Wout, bout, gout, beout):
    from concourse.bass_utils import run_bass_kernel_spmd

    x = np.asarray(x, np.float32)
    mask = np.asarray(mask)
    sc = 1.0 / float(np.sqrt(np.float32(DK)))

    z = lambda a: not np.any(np.asarray(a))
    one = lambda a: bool(np.all(np.asarray(a) == 1.0))
    flags = (not z(bq), not z(bk), not z(bv), not z(bo), not z(bso),
             not z(bi), not z(bout),
             not one(gso), not z(beso), not one(gout), not z(beout))
    nc = _get_program(flags)

    wq_b = _bf(np.asarray(Wq, np.float32) * sc)
    wk_b, wv_b, wo_b, wso_b = _bf(Wk), _bf(Wv), _bf(Wo), _bf(Wso)
    wi_b, wout_b = _bf(Wi), _bf(Wout)
    identb = _bf(np.eye(128))
    onesr = _bf(np.ones((1, 512)))

    brow = np.zeros((7, FF), np.float32)
    brow[0, :E] = np.asarray(bq, np.float32) * sc
    for i, v in enumerate((bk, bv, bo, bso, bout)):
        brow[i + 1, :E] = v
    brow[6, :] = bi
    brow = _bf(brow)
    bicol = np.asarray(bi, np.float32).reshape(FF // 128, 128).T.copy()
    gbt = np.zeros((128, 4 * E), np.float32)
    for i, g in enumerate((gso, gout, beso, beout)):   # gamma1|gamma2|beta1|beta2
        gbt[:, i * E:(i + 1) * E] = np.broadcast_to(
            np.asarray(g, np.float32).reshape(1, E), (128, E))

    in_maps = []
    for c in range(NCORES):
        xs = np.ascontiguousarray(x[c * BL:(c + 1) * BL].reshape(T, E).T)
        ms = np.asarray(mask[c * BL:(c + 1) * BL]).reshape(BL, S)
        mb = np.where(ms == 0, np.float32(MASK_NEG), np.float32(0.0))
        # mcol[p, b*4+jt] = mask bias of kv position jt*128+p in batch b
        mcolv = np.ascontiguousarray(
            mb.reshape(BL, S // 128, 128).transpose(2, 0, 1).reshape(
                128, BL * (S // 128)).astype(np.float32))
        in_maps.append({
            "x": _bf(xs), "wq": wq_b, "wk": wk_b, "wv": wv_b, "wo": wo_b,
            "wso": wso_b, "wi": wi_b, "wout": wout_b, "mcol": mcolv,
            "ident": identb, "onesrow": onesr,
            "brow": brow, "bicol": bicol, "gb": gbt,
        })

    trace = os.environ.get("KERNEL_TRACE", "0") == "1"
    res = run_bass_kernel_spmd(nc, in_maps, core_ids=list(range(NCORES)),
                               trace=trace)
    if trace and res.exec_time_ns is not None:
        print(f"HW exec time: {res.exec_time_ns} ns")
        if res.instructions_and_trace is not None:
            print(f"trace: {res.instructions_and_trace[1]}")
    out = np.concatenate([r["out"].reshape(BL, S, E) for r in res.results],
                         axis=0)
    return np.ascontiguousarray(out.astype(np.float32))
